# revision 4
# baseline (speedup 1.0000x reference)
"""Trainium2 Bass kernel for cross-attention (fp16, true-max softmax):
    scores  = dec @ enc^T            [B, Tq, Tk]
    probs   = softmax(scores, -1)
    context = probs @ enc            [B, Tq, D]

Shapes (hardcoded): enc [16, 2048, 1024] f32, dec [16, 128, 1024] f32.
Sharding: data-parallel over batch B across 8 NeuronCores (2 batches/core).

Design:
  - enc/dec stream in as fp16 via gpsimd casting DMAs (HBM still reads
    f32; SBUF holds f16) -> PE transposes run at 1.0 cyc/col instead of
    1.5, PSUM->SBUF copies halve, and mm1/mm2 run f16 x f16 -> f32 PSUM.
  - Two-phase softmax per batch: all 4 mm1 chunks first (scores copied
    PSUM -> SBUF f32), per-chunk row-max on DVE, then ONE exact shift
    bias = -rowmax(all 2048) for the exps. exp <= 1 always, so f16
    probs cannot overflow (a streamed chunk-0-max shift can be beaten
    by ~36+ in later chunks, which overflows f16's e^11 range).
  - PE order interleaves batch-1 A-phases with batch-0 B-phases to keep
    the softmax latency off the critical path; batch-0 B3 slots between
    batch-1 A3 and batch-1 B0 to hide batch-1's max+exp latency.
  - All input DMAs dispatched up-front in arrival order on the SWDGE
    queue; warmup junk matmuls hold the PE HAM clock at speed until
    real work starts; identity built before the DMA preps so nothing
    queues behind them on the Pool engine.
  - Last chunk's exp/probsT/mm2 split in two halves; output scales
    split in 256-col quarters across ACT/DVE, stores on sync/act HWDGE.
"""

import sys

sys.path.insert(0, "/opt/trn_rl_repo")

import numpy as np
from contextlib import ExitStack

import concourse.bass as bass
import concourse.tile as tile
from concourse import bacc, mybir
from concourse.masks import make_identity

F32 = mybir.dt.float32
F16 = mybir.dt.float16
EXP = mybir.ActivationFunctionType.Exp
COPY = mybir.ActivationFunctionType.Copy
AX_X = mybir.AxisListType.X

B, Tk, Tq, D = 16, 2048, 128, 1024
CORES = 8
BLOC = B // CORES          # batches per core
KCH = 4                    # k chunks per batch
KCS = Tk // KCH            # 512 k rows per chunk
NSUB = KCS // 128          # 4 k-subtiles per chunk
DT = D // 128              # 8 d-tiles
DH = D // 512              # 2 output column halves
NCHUNK = BLOC * KCH

NWARM = 24                 # junk matmuls bridging the HAM clock ramp
WARM_N = 64

_CACHE = {}


def _build(nwarm=None):
    nwarm = NWARM if nwarm is None else nwarm
    nc = bacc.Bacc("TRN2", debug=False, num_devices=CORES)
    enc = nc.dram_tensor("enc", [BLOC, Tk, D], F32, kind="ExternalInput").ap()
    dec = nc.dram_tensor("dec", [BLOC, Tq, D], F32, kind="ExternalInput").ap()
    out = nc.dram_tensor("out", [BLOC, Tq, D], F32, kind="ExternalOutput").ap()

    with tile.TileContext(nc) as tc, ExitStack() as ctx:
        sb = ctx.enter_context(tc.tile_pool(name="sb", bufs=1))
        enc_p = ctx.enter_context(tc.tile_pool(name="enc", bufs=8))
        encT_p = ctx.enter_context(tc.tile_pool(name="encT", bufs=10))
        dec_p = ctx.enter_context(tc.tile_pool(name="dec", bufs=2))
        decT_p = ctx.enter_context(tc.tile_pool(name="decT", bufs=4))
        probs_p = ctx.enter_context(tc.tile_pool(name="probs", bufs=2))
        probsT_p = ctx.enter_context(tc.tile_pool(name="probsT", bufs=8))
        scall_p = ctx.enter_context(tc.tile_pool(name="scall", bufs=2))
        outp_p = ctx.enter_context(tc.tile_pool(name="outp", bufs=2))
        stat_p = ctx.enter_context(tc.tile_pool(name="stat", bufs=4))
        sc_p = ctx.enter_context(tc.tile_pool(name="sc", bufs=2, space="PSUM"))
        tr_p = ctx.enter_context(tc.tile_pool(name="tr", bufs=4, space="PSUM"))
        ctx_p = ctx.enter_context(tc.tile_pool(name="ctx", bufs=2, space="PSUM"))

        # ---- HAM warm-up + identity, before any SWDGE prep hits Pool
        junk = sb.tile([128, 128], F32)
        nc.gpsimd.memset(junk[:], 0.0)
        warm = sc_p.tile([128, 512], F32, tag="sc", name="warm")
        for i in range(nwarm):
            nc.tensor.matmul(
                warm[:, 0:WARM_N], junk[:], junk[:, 0:WARM_N],
                start=(i == 0), stop=(i == nwarm - 1),
            )

        ident = sb.tile([128, 128], F32)
        ident16 = sb.tile([128, 128], F16)
        make_identity(nc, ident[:])
        nc.vector.tensor_copy(ident16[:], ident[:])
        zeros = sb.tile([128, KCS], F32)
        nc.gpsimd.memset(zeros[:], 0.0)

        # ---- all input DMAs up-front (SWDGE f32 -> f16 casts)
        dec_sb = {}
        enc_sb_all = {}

        def dma_dec(b):
            t = dec_p.tile([128, D], F16, tag="dec", name=f"dec{b}")
            nc.gpsimd.dma_start(t[:], dec[b])
            dec_sb[b] = t

        def dma_chunk(g, pieces=1):
            b, kc = divmod(g, KCH)
            et = enc_p.tile([128, NSUB, D], F16, tag="enc", name=f"enc{g}")
            enc_sb_all[g] = et
            # SWDGE prep cost is ~flat per instruction: full chunks keep
            # Pool light; chunk 0 is split so the head starts early
            w = D // pieces
            for i in range(pieces):
                nc.gpsimd.dma_start(
                    et[:, :, w * i : w * (i + 1)],
                    enc[b, kc * KCS : (kc + 1) * KCS, w * i : w * (i + 1)]
                    .rearrange("(n p) d -> p n d", p=128),
                )

        dma_dec(0)
        dma_chunk(0, pieces=4)
        dma_chunk(1, pieces=2)
        dma_dec(1)
        for g in range(2, NCHUNK):
            dma_chunk(g, pieces=2)

        copy_count = [0]

        def psum2sbuf(dst, src, engines="both"):
            if engines == "dve":
                nc.vector.tensor_copy(dst, src)
                return
            if copy_count[0] % 2 == 0:
                nc.vector.tensor_copy(dst, src)
            else:
                nc.scalar.copy(dst, src)
            copy_count[0] += 1

        state = {}

        def begin_batch(b):
            st = {}
            nsum = KCH + 1 if b == BLOC - 1 else KCH
            st["negsh"] = stat_p.tile([128, 1], F32, tag="negsh", name=f"ns{b}")
            st["rm"] = stat_p.tile([128, KCH], F32, tag="rm", name=f"rm{b}")
            st["probs"] = probs_p.tile([128, Tk], F16, tag="probs", name=f"pr{b}")
            st["sums"] = stat_p.tile([128, nsum], F32, tag="sums", name=f"sm{b}")
            st["scall"] = scall_p.tile([128, Tk], F32, tag="scall", name=f"sa{b}")
            st["cps"] = [
                ctx_p.tile([128, 512], F32, tag="ctx", name=f"cps{b}_{dh}")
                for dh in range(DH)
            ]
            state[b] = st

        def emit_decT(b):
            st = state[b]
            decb = dec_sb[b]
            decT = []
            for blk in range(2):
                trt = tr_p.tile([128, 1024], F16, tag="tr", name=f"trd{b}_{blk}")[
                    :, 0:512
                ]
                for j in range(4):
                    dd = 4 * blk + j
                    nc.tensor.transpose(
                        trt[:, 128 * j : 128 * (j + 1)],
                        decb[:, 128 * dd : 128 * (dd + 1)],
                        ident16[:],
                    )
                dstT = decT_p.tile([128, 512], F16, tag="decT", name=f"dT{b}_{blk}")
                psum2sbuf(dstT[:], trt[:])
                decT.append(dstT)
            st["decT"] = decT

        def stage_a(g, with_decT=False):
            """transposes + mm1 + row-max + scores PSUM->SBUF copy."""
            b, kc = divmod(g, KCH)
            st = state[b]
            et = enc_sb_all[g]
            scores = sc_p.tile([128, KCS], F32, tag="sc", name=f"sc{g}")
            encT = {}
            pend = []

            def mm1(dd):
                nc.tensor.matmul(
                    scores[:],
                    st["decT"][dd // 4][:, 128 * (dd % 4) : 128 * (dd % 4 + 1)],
                    encT[dd][:],
                    start=(dd == 0),
                    stop=(dd == DT - 1),
                )

            for dp in range(DT // 2):
                trt = tr_p.tile([128, 1024], F16, tag="tr", name=f"tr{g}_{dp}")
                for h2 in range(2):
                    d = 2 * dp + h2
                    for n in range(NSUB):
                        nc.tensor.transpose(
                            trt[:, 512 * h2 + 128 * n : 512 * h2 + 128 * (n + 1)],
                            et[:, n, 128 * d : 128 * (d + 1)],
                            ident16[:],
                        )
                eT = encT_p.tile([128, 1024], F16, tag="encT", name=f"eT{g}_{dp}")
                psum2sbuf(eT[:], trt[:])
                encT[2 * dp] = eT[:, 0:512]
                encT[2 * dp + 1] = eT[:, 512:1024]
                pend.extend([2 * dp, 2 * dp + 1])
                if with_decT and dp == 2:
                    emit_decT(b)
                if dp >= 1 and (not with_decT or dp >= 3):
                    mm1(pend.pop(0))
                    mm1(pend.pop(0))
            while pend:
                mm1(pend.pop(0))

            nc.vector.reduce_max(st["rm"][:, kc : kc + 1], scores[:], axis=AX_X)
            nc.vector.tensor_copy(
                st["scall"][:, kc * KCS : (kc + 1) * KCS], scores[:]
            )

        def emit_negsh_exps(b, split_last=False):
            st = state[b]
            # exact shift: -(row max over all 2048)
            nc.vector.reduce_max(st["negsh"][:], st["rm"][:], axis=AX_X, negate=True)
            pieces = [(0, 512), (512, 1024), (1024, 1536)]
            if split_last:
                pieces += [(1536, 1792), (1792, 2048)]
            else:
                pieces += [(1536, 2048)]
            for i, (c0, c1) in enumerate(pieces):
                nc.scalar.activation(
                    st["probs"][:, c0:c1],
                    st["scall"][:, c0:c1],
                    EXP,
                    bias=st["negsh"][:],
                    scale=1.0,
                    accum_out=st["sums"][:, i : i + 1],
                )

        def stage_b(g, pjs=None, h=0, copy_eng="both"):
            """probsT + mm2 for k-subtiles pjs (default all 4) of chunk g."""
            b, kc = divmod(g, KCH)
            st = state[b]
            if pjs is None:
                pjs = [0, 1, 2, 3]
            trt = tr_p.tile([128, 1024], F16, tag="tr", name=f"trp{g}_{h}")[
                :, 0 : 128 * len(pjs)
            ]
            for i, j in enumerate(pjs):
                t = 4 * kc + j
                nc.tensor.transpose(
                    trt[:, 128 * i : 128 * (i + 1)],
                    st["probs"][:, 128 * t : 128 * (t + 1)],
                    ident16[:],
                )
            pT = probsT_p.tile(
                [128, 128 * len(pjs)], F16, tag="probsT", name=f"pT{g}_{h}"
            )
            psum2sbuf(pT[:], trt[:], engines=copy_eng)
            et = enc_sb_all[g]
            last = g == NCHUNK - 1
            order = (
                [(dh, i) for dh in range(DH) for i in range(len(pjs))]
                if last
                else [(dh, i) for i in range(len(pjs)) for dh in range(DH)]
            )
            for dh, i in order:
                t = 4 * kc + pjs[i]
                nc.tensor.matmul(
                    st["cps"][dh][:],
                    pT[:, 128 * i : 128 * (i + 1)],
                    et[:, pjs[i], dh * 512 : (dh + 1) * 512],
                    start=(t == 0),
                    stop=(t == 4 * KCH - 1),
                )

        def finish_batch(b):
            st = state[b]
            denom = stat_p.tile([128, 1], F32, tag="denom", name=f"dn{b}")
            nc.vector.reduce_sum(denom[:], st["sums"][:], axis=AX_X)
            rdenom = stat_p.tile([128, 1], F32, tag="rdenom", name=f"rd{b}")
            nc.vector.reciprocal(rdenom[:], denom[:])
            out_sb = outp_p.tile([128, D], F32, tag="outp", name=f"ou{b}")
            if b == BLOC - 1:
                # one full-half scale per engine (parallel), 2 DMAs on
                # separate HWDGE queues: shortest store chain
                nc.scalar.activation(
                    out_sb[:, 0:512], st["cps"][0][:], COPY, bias=0.0,
                    scale=rdenom[:],
                )
                nc.vector.tensor_scalar_mul(
                    out_sb[:, 512:1024], st["cps"][1][:], rdenom[:],
                )
                nc.sync.dma_start(out[b][:, 0:512], out_sb[:, 0:512])
                nc.scalar.dma_start(out[b][:, 512:1024], out_sb[:, 512:1024])
            else:
                nc.vector.tensor_scalar_mul(
                    out_sb[:, 0:512], st["cps"][0][:], rdenom[:],
                )
                nc.vector.tensor_scalar_mul(
                    out_sb[:, 512:1024], st["cps"][1][:], rdenom[:],
                )
                nc.sync.dma_start(out[b], out_sb[:])

        # ---- PE program
        begin_batch(0)
        emit_decT(0)
        stage_a(0)
        stage_a(1)
        stage_a(2)
        stage_a(3)
        emit_negsh_exps(0)
        begin_batch(1)
        stage_a(4, with_decT=True)
        stage_a(5)
        stage_b(0)
        stage_a(6)
        stage_b(1)
        stage_a(7)
        stage_b(2)
        emit_negsh_exps(1, split_last=True)
        stage_b(3, copy_eng="dve")
        finish_batch(0)
        stage_b(4, copy_eng="dve")
        stage_b(5, copy_eng="dve")
        stage_b(6, copy_eng="dve")
        stage_b(7, pjs=[0, 1], h=0, copy_eng="dve")
        stage_b(7, pjs=[2, 3], h=1, copy_eng="dve")
        finish_batch(1)

    nc.compile()
    return nc


def kernel(encoder_hiddens: np.ndarray, decoder_hidden: np.ndarray) -> np.ndarray:
    enc = np.ascontiguousarray(np.asarray(encoder_hiddens, dtype=np.float32))
    dec = np.ascontiguousarray(np.asarray(decoder_hidden, dtype=np.float32))
    assert enc.shape == (B, Tk, D) and dec.shape == (B, Tq, D)

    if "nc" not in _CACHE:
        _CACHE["nc"] = _build()
    nc = _CACHE["nc"]

    from concourse.bass_utils import run_bass_kernel_spmd

    in_maps = [
        {
            "enc": enc[c * BLOC : (c + 1) * BLOC],
            "dec": dec[c * BLOC : (c + 1) * BLOC],
        }
        for c in range(CORES)
    ]
    res = None
    for attempt in range(3):
        try:
            res = run_bass_kernel_spmd(nc, in_maps, core_ids=list(range(CORES)))
            break
        except Exception:
            if attempt == 2:
                raise
            import time

            time.sleep(15)
    out = np.empty((B, Tq, D), dtype=np.float32)
    for c in range(CORES):
        out[c * BLOC : (c + 1) * BLOC] = res.results[c]["out"]
    return out


# revision 5
# speedup vs baseline: 1.0284x; 1.0284x over previous
"""Trainium2 Bass kernel for cross-attention (fp16, true-max softmax):
    scores  = dec @ enc^T            [B, Tq, Tk]
    probs   = softmax(scores, -1)
    context = probs @ enc            [B, Tq, D]

Shapes (hardcoded): enc [16, 2048, 1024] f32, dec [16, 128, 1024] f32.
Sharding: data-parallel over batch B across 8 NeuronCores (2 batches/core).

Design:
  - enc/dec stream in as fp16 via gpsimd casting DMAs (HBM still reads
    f32; SBUF holds f16) -> PE transposes run at 1.0 cyc/col instead of
    1.5, PSUM->SBUF copies halve, and mm1/mm2 run f16 x f16 -> f32 PSUM.
  - Two-phase softmax per batch: all 4 mm1 chunks first (scores copied
    PSUM -> SBUF f32), per-chunk row-max on DVE, then ONE exact shift
    bias = -rowmax(all 2048) for the exps. exp <= 1 always, so f16
    probs cannot overflow (a streamed chunk-0-max shift can be beaten
    by ~36+ in later chunks, which overflows f16's e^11 range).
  - PE order interleaves batch-1 A-phases 2:1 ahead of batch-0
    B-phases (A4 A5 A6 B0 A7 B1 | exps1 | B2 B3 finish0 B4..B7): both
    batches' max+exp chains then resolve entirely under PE fill, and
    the two-phase softmax (plain DVE copy + reduce_max; the fused
    tensor_tensor_reduce lowers to custom DVE ucode that faults the
    device) costs no PE stall.
  - All input DMAs dispatched up-front in arrival order on the SWDGE
    queue; warmup junk matmuls hold the PE HAM clock at speed until
    real work starts; identity built before the DMA preps so nothing
    queues behind them on the Pool engine.
  - Last chunk's exp/probsT/mm2 split in two halves; output scales
    split in 256-col quarters across ACT/DVE, stores on sync/act HWDGE.
"""

import sys

sys.path.insert(0, "/opt/trn_rl_repo")

import numpy as np
from contextlib import ExitStack

import concourse.bass as bass
import concourse.tile as tile
from concourse import bacc, mybir
from concourse.masks import make_identity

F32 = mybir.dt.float32
F16 = mybir.dt.float16
EXP = mybir.ActivationFunctionType.Exp
COPY = mybir.ActivationFunctionType.Copy
AX_X = mybir.AxisListType.X

B, Tk, Tq, D = 16, 2048, 128, 1024
CORES = 8
BLOC = B // CORES          # batches per core
KCH = 4                    # k chunks per batch
KCS = Tk // KCH            # 512 k rows per chunk
NSUB = KCS // 128          # 4 k-subtiles per chunk
DT = D // 128              # 8 d-tiles
DH = D // 512              # 2 output column halves
NCHUNK = BLOC * KCH

NWARM = 29                 # junk matmuls bridging the HAM clock ramp
WARM_N = 48

_CACHE = {}


def _build(nwarm=None):
    nwarm = NWARM if nwarm is None else nwarm
    nc = bacc.Bacc("TRN2", debug=False, num_devices=CORES)
    enc = nc.dram_tensor("enc", [BLOC, Tk, D], F32, kind="ExternalInput").ap()
    dec = nc.dram_tensor("dec", [BLOC, Tq, D], F32, kind="ExternalInput").ap()
    out = nc.dram_tensor("out", [BLOC, Tq, D], F32, kind="ExternalOutput").ap()

    with tile.TileContext(nc) as tc, ExitStack() as ctx:
        sb = ctx.enter_context(tc.tile_pool(name="sb", bufs=1))
        enc_p = ctx.enter_context(tc.tile_pool(name="enc", bufs=8))
        encT_p = ctx.enter_context(tc.tile_pool(name="encT", bufs=10))
        dec_p = ctx.enter_context(tc.tile_pool(name="dec", bufs=2))
        decT_p = ctx.enter_context(tc.tile_pool(name="decT", bufs=4))
        probs_p = ctx.enter_context(tc.tile_pool(name="probs", bufs=2))
        probsT_p = ctx.enter_context(tc.tile_pool(name="probsT", bufs=8))
        scall_p = ctx.enter_context(tc.tile_pool(name="scall", bufs=2))
        outp_p = ctx.enter_context(tc.tile_pool(name="outp", bufs=2))
        stat_p = ctx.enter_context(tc.tile_pool(name="stat", bufs=4))
        sc_p = ctx.enter_context(tc.tile_pool(name="sc", bufs=2, space="PSUM"))
        tr_p = ctx.enter_context(tc.tile_pool(name="tr", bufs=4, space="PSUM"))
        ctx_p = ctx.enter_context(tc.tile_pool(name="ctx", bufs=2, space="PSUM"))

        # ---- HAM warm-up + identity, before any SWDGE prep hits Pool
        junk = sb.tile([128, 128], F32)
        nc.gpsimd.memset(junk[:], 0.0)
        warm = sc_p.tile([128, 512], F32, tag="sc", name="warm")
        for i in range(nwarm):
            nc.tensor.matmul(
                warm[:, 0:WARM_N], junk[:], junk[:, 0:WARM_N],
                start=(i == 0), stop=(i == nwarm - 1),
            )

        ident = sb.tile([128, 128], F32)
        ident16 = sb.tile([128, 128], F16)
        make_identity(nc, ident[:])
        nc.vector.tensor_copy(ident16[:], ident[:])
        zeros = sb.tile([128, KCS], F32)
        nc.gpsimd.memset(zeros[:], 0.0)

        # ---- all input DMAs up-front (SWDGE f32 -> f16 casts)
        dec_sb = {}
        enc_sb_all = {}

        def dma_dec(b):
            t = dec_p.tile([128, D], F16, tag="dec", name=f"dec{b}")
            nc.gpsimd.dma_start(t[:], dec[b])
            dec_sb[b] = t

        def dma_chunk(g, pieces=1):
            b, kc = divmod(g, KCH)
            et = enc_p.tile([128, NSUB, D], F16, tag="enc", name=f"enc{g}")
            enc_sb_all[g] = et
            # SWDGE prep cost is ~flat per instruction: full chunks keep
            # Pool light; chunk 0 is split so the head starts early
            w = D // pieces
            for i in range(pieces):
                nc.gpsimd.dma_start(
                    et[:, :, w * i : w * (i + 1)],
                    enc[b, kc * KCS : (kc + 1) * KCS, w * i : w * (i + 1)]
                    .rearrange("(n p) d -> p n d", p=128),
                )

        dma_dec(0)
        dma_chunk(0, pieces=4)
        dma_chunk(1, pieces=2)
        dma_dec(1)
        for g in range(2, NCHUNK):
            dma_chunk(g, pieces=2)

        copy_count = [0]

        def psum2sbuf(dst, src, engines="both"):
            if engines == "dve":
                nc.vector.tensor_copy(dst, src)
                return
            if copy_count[0] % 2 == 0:
                nc.vector.tensor_copy(dst, src)
            else:
                nc.scalar.copy(dst, src)
            copy_count[0] += 1

        state = {}

        def begin_batch(b):
            st = {}
            nsum = KCH + 1 if b == BLOC - 1 else KCH
            st["negsh"] = stat_p.tile([128, 1], F32, tag="negsh", name=f"ns{b}")
            st["rm"] = stat_p.tile([128, KCH], F32, tag="rm", name=f"rm{b}")
            st["probs"] = probs_p.tile([128, Tk], F16, tag="probs", name=f"pr{b}")
            st["sums"] = stat_p.tile([128, nsum], F32, tag="sums", name=f"sm{b}")
            st["scall"] = scall_p.tile([128, Tk], F32, tag="scall", name=f"sa{b}")
            st["cps"] = [
                ctx_p.tile([128, 512], F32, tag="ctx", name=f"cps{b}_{dh}")
                for dh in range(DH)
            ]
            state[b] = st

        def emit_decT(b):
            st = state[b]
            decb = dec_sb[b]
            decT = []
            for blk in range(2):
                trt = tr_p.tile([128, 1024], F16, tag="tr", name=f"trd{b}_{blk}")[
                    :, 0:512
                ]
                for j in range(4):
                    dd = 4 * blk + j
                    nc.tensor.transpose(
                        trt[:, 128 * j : 128 * (j + 1)],
                        decb[:, 128 * dd : 128 * (dd + 1)],
                        ident16[:],
                    )
                dstT = decT_p.tile([128, 512], F16, tag="decT", name=f"dT{b}_{blk}")
                psum2sbuf(dstT[:], trt[:])
                decT.append(dstT)
            st["decT"] = decT

        def stage_a(g, with_decT=False):
            """transposes + mm1 + row-max + scores PSUM->SBUF copy."""
            b, kc = divmod(g, KCH)
            st = state[b]
            et = enc_sb_all[g]
            scores = sc_p.tile([128, KCS], F32, tag="sc", name=f"sc{g}")
            encT = {}
            pend = []

            def mm1(dd):
                nc.tensor.matmul(
                    scores[:],
                    st["decT"][dd // 4][:, 128 * (dd % 4) : 128 * (dd % 4 + 1)],
                    encT[dd][:],
                    start=(dd == 0),
                    stop=(dd == DT - 1),
                )

            for dp in range(DT // 2):
                trt = tr_p.tile([128, 1024], F16, tag="tr", name=f"tr{g}_{dp}")
                for h2 in range(2):
                    d = 2 * dp + h2
                    for n in range(NSUB):
                        nc.tensor.transpose(
                            trt[:, 512 * h2 + 128 * n : 512 * h2 + 128 * (n + 1)],
                            et[:, n, 128 * d : 128 * (d + 1)],
                            ident16[:],
                        )
                eT = encT_p.tile([128, 1024], F16, tag="encT", name=f"eT{g}_{dp}")
                psum2sbuf(eT[:], trt[:])
                encT[2 * dp] = eT[:, 0:512]
                encT[2 * dp + 1] = eT[:, 512:1024]
                pend.extend([2 * dp, 2 * dp + 1])
                if with_decT and dp == 2:
                    emit_decT(b)
                if dp >= 1 and (not with_decT or dp >= 3):
                    mm1(pend.pop(0))
                    mm1(pend.pop(0))
            while pend:
                mm1(pend.pop(0))

            nc.vector.reduce_max(st["rm"][:, kc : kc + 1], scores[:], axis=AX_X)
            nc.vector.tensor_copy(
                st["scall"][:, kc * KCS : (kc + 1) * KCS], scores[:]
            )

        def emit_negsh_exps(b, split_last=False):
            st = state[b]
            # exact shift: -(row max over all 2048)
            nc.vector.reduce_max(st["negsh"][:], st["rm"][:], axis=AX_X, negate=True)
            pieces = [(0, 512), (512, 1024), (1024, 1536)]
            if split_last:
                pieces += [(1536, 1792), (1792, 2048)]
            else:
                pieces += [(1536, 2048)]
            for i, (c0, c1) in enumerate(pieces):
                nc.scalar.activation(
                    st["probs"][:, c0:c1],
                    st["scall"][:, c0:c1],
                    EXP,
                    bias=st["negsh"][:],
                    scale=1.0,
                    accum_out=st["sums"][:, i : i + 1],
                )

        def stage_b(g, pjs=None, h=0, copy_eng="both"):
            """probsT + mm2 for k-subtiles pjs (default all 4) of chunk g."""
            b, kc = divmod(g, KCH)
            st = state[b]
            if pjs is None:
                pjs = [0, 1, 2, 3]
            trt = tr_p.tile([128, 1024], F16, tag="tr", name=f"trp{g}_{h}")[
                :, 0 : 128 * len(pjs)
            ]
            for i, j in enumerate(pjs):
                t = 4 * kc + j
                nc.tensor.transpose(
                    trt[:, 128 * i : 128 * (i + 1)],
                    st["probs"][:, 128 * t : 128 * (t + 1)],
                    ident16[:],
                )
            pT = probsT_p.tile(
                [128, 128 * len(pjs)], F16, tag="probsT", name=f"pT{g}_{h}"
            )
            psum2sbuf(pT[:], trt[:], engines=copy_eng)
            et = enc_sb_all[g]
            last = g == NCHUNK - 1
            order = (
                [(dh, i) for dh in range(DH) for i in range(len(pjs))]
                if last
                else [(dh, i) for i in range(len(pjs)) for dh in range(DH)]
            )
            for dh, i in order:
                t = 4 * kc + pjs[i]
                nc.tensor.matmul(
                    st["cps"][dh][:],
                    pT[:, 128 * i : 128 * (i + 1)],
                    et[:, pjs[i], dh * 512 : (dh + 1) * 512],
                    start=(t == 0),
                    stop=(t == 4 * KCH - 1),
                )

        def finish_batch(b):
            st = state[b]
            denom = stat_p.tile([128, 1], F32, tag="denom", name=f"dn{b}")
            nc.vector.reduce_sum(denom[:], st["sums"][:], axis=AX_X)
            rdenom = stat_p.tile([128, 1], F32, tag="rdenom", name=f"rd{b}")
            nc.vector.reciprocal(rdenom[:], denom[:])
            out_sb = outp_p.tile([128, D], F32, tag="outp", name=f"ou{b}")
            if b == BLOC - 1:
                # one full-half scale per engine (parallel), 2 DMAs on
                # separate HWDGE queues: shortest store chain
                nc.scalar.activation(
                    out_sb[:, 0:512], st["cps"][0][:], COPY, bias=0.0,
                    scale=rdenom[:],
                )
                nc.vector.tensor_scalar_mul(
                    out_sb[:, 512:1024], st["cps"][1][:], rdenom[:],
                )
                nc.sync.dma_start(out[b][:, 0:512], out_sb[:, 0:512])
                nc.scalar.dma_start(out[b][:, 512:1024], out_sb[:, 512:1024])
            else:
                nc.vector.tensor_scalar_mul(
                    out_sb[:, 0:512], st["cps"][0][:], rdenom[:],
                )
                nc.vector.tensor_scalar_mul(
                    out_sb[:, 512:1024], st["cps"][1][:], rdenom[:],
                )
                nc.sync.dma_start(out[b], out_sb[:])

        # ---- PE program
        begin_batch(0)
        emit_decT(0)
        stage_a(0)
        stage_a(1)
        stage_a(2)
        stage_a(3)
        emit_negsh_exps(0)
        begin_batch(1)
        stage_a(4, with_decT=True)
        stage_a(5)
        stage_a(6)
        stage_b(0)
        stage_a(7)
        stage_b(1)
        emit_negsh_exps(1, split_last=True)
        stage_b(2, copy_eng="dve")
        stage_b(3, copy_eng="dve")
        finish_batch(0)
        stage_b(4, copy_eng="dve")
        stage_b(5, copy_eng="dve")
        stage_b(6, copy_eng="dve")
        stage_b(7, pjs=[0, 1], h=0, copy_eng="dve")
        stage_b(7, pjs=[2, 3], h=1, copy_eng="dve")
        finish_batch(1)

    nc.compile()
    return nc


def kernel(encoder_hiddens: np.ndarray, decoder_hidden: np.ndarray) -> np.ndarray:
    enc = np.ascontiguousarray(np.asarray(encoder_hiddens, dtype=np.float32))
    dec = np.ascontiguousarray(np.asarray(decoder_hidden, dtype=np.float32))
    assert enc.shape == (B, Tk, D) and dec.shape == (B, Tq, D)

    if "nc" not in _CACHE:
        _CACHE["nc"] = _build()
    nc = _CACHE["nc"]

    from concourse.bass_utils import run_bass_kernel_spmd

    in_maps = [
        {
            "enc": enc[c * BLOC : (c + 1) * BLOC],
            "dec": dec[c * BLOC : (c + 1) * BLOC],
        }
        for c in range(CORES)
    ]
    res = None
    for attempt in range(3):
        try:
            res = run_bass_kernel_spmd(nc, in_maps, core_ids=list(range(CORES)))
            break
        except Exception:
            if attempt == 2:
                raise
            import time

            time.sleep(15)
    out = np.empty((B, Tq, D), dtype=np.float32)
    for c in range(CORES):
        out[c * BLOC : (c + 1) * BLOC] = res.results[c]["out"]
    return out


# revision 6
# speedup vs baseline: 1.0320x; 1.0035x over previous
"""Trainium2 Bass kernel for cross-attention (fp16, true-max softmax):
    scores  = dec @ enc^T            [B, Tq, Tk]
    probs   = softmax(scores, -1)
    context = probs @ enc            [B, Tq, D]

Shapes (hardcoded): enc [16, 2048, 1024] f32, dec [16, 128, 1024] f32.
Sharding: data-parallel over batch B across 8 NeuronCores (2 batches/core).

Design:
  - enc/dec stream in as fp16 via gpsimd casting DMAs (HBM still reads
    f32; SBUF holds f16) -> PE transposes run at 1.0 cyc/col instead of
    1.5, PSUM->SBUF copies halve, and mm1/mm2 run f16 x f16 -> f32 PSUM.
  - Two-phase softmax per batch: all 4 mm1 chunks first (scores copied
    PSUM -> SBUF f32), per-chunk row-max on DVE, then ONE exact shift
    bias = -rowmax(all 2048) for the exps. exp <= 1 always, so f16
    probs cannot overflow (a streamed chunk-0-max shift can be beaten
    by ~36+ in later chunks, which overflows f16's e^11 range).
  - PE order interleaves batch-1 A-phases 2:1 ahead of batch-0
    B-phases (A4 A5 A6 B0 A7 B1 | exps1 | B2 B3 finish0 B4..B7): both
    batches' max+exp chains then resolve entirely under PE fill, and
    the two-phase softmax (plain DVE copy + reduce_max; the fused
    tensor_tensor_reduce lowers to custom DVE ucode that faults the
    device) costs no PE stall.
  - All input DMAs dispatched up-front in arrival order on the SWDGE
    queue; warmup junk matmuls hold the PE HAM clock at speed until
    real work starts; identity built before the DMA preps so nothing
    queues behind them on the Pool engine.
  - Last chunk's mm2 runs fully dh-major so cps[0] closes 4 matmuls
    early: its scale+store clears the DMA engines before cps[1]'s
    store arrives. Output halves scale on ACT/DVE in parallel and
    store via sync/act HWDGE queues.
"""

import sys

sys.path.insert(0, "/opt/trn_rl_repo")

import numpy as np
from contextlib import ExitStack

import concourse.bass as bass
import concourse.tile as tile
from concourse import bacc, mybir
from concourse.masks import make_identity

F32 = mybir.dt.float32
F16 = mybir.dt.float16
EXP = mybir.ActivationFunctionType.Exp
COPY = mybir.ActivationFunctionType.Copy
AX_X = mybir.AxisListType.X

B, Tk, Tq, D = 16, 2048, 128, 1024
CORES = 8
BLOC = B // CORES          # batches per core
KCH = 4                    # k chunks per batch
KCS = Tk // KCH            # 512 k rows per chunk
NSUB = KCS // 128          # 4 k-subtiles per chunk
DT = D // 128              # 8 d-tiles
DH = D // 512              # 2 output column halves
NCHUNK = BLOC * KCH

NWARM = 29                 # junk matmuls bridging the HAM clock ramp
WARM_N = 48

_CACHE = {}


def _build(nwarm=None):
    nwarm = NWARM if nwarm is None else nwarm
    nc = bacc.Bacc("TRN2", debug=False, num_devices=CORES)
    enc = nc.dram_tensor("enc", [BLOC, Tk, D], F32, kind="ExternalInput").ap()
    dec = nc.dram_tensor("dec", [BLOC, Tq, D], F32, kind="ExternalInput").ap()
    out = nc.dram_tensor("out", [BLOC, Tq, D], F32, kind="ExternalOutput").ap()

    with tile.TileContext(nc) as tc, ExitStack() as ctx:
        sb = ctx.enter_context(tc.tile_pool(name="sb", bufs=1))
        enc_p = ctx.enter_context(tc.tile_pool(name="enc", bufs=8))
        encT_p = ctx.enter_context(tc.tile_pool(name="encT", bufs=10))
        dec_p = ctx.enter_context(tc.tile_pool(name="dec", bufs=2))
        decT_p = ctx.enter_context(tc.tile_pool(name="decT", bufs=4))
        probs_p = ctx.enter_context(tc.tile_pool(name="probs", bufs=2))
        probsT_p = ctx.enter_context(tc.tile_pool(name="probsT", bufs=8))
        scall_p = ctx.enter_context(tc.tile_pool(name="scall", bufs=2))
        outp_p = ctx.enter_context(tc.tile_pool(name="outp", bufs=2))
        stat_p = ctx.enter_context(tc.tile_pool(name="stat", bufs=4))
        sc_p = ctx.enter_context(tc.tile_pool(name="sc", bufs=2, space="PSUM"))
        tr_p = ctx.enter_context(tc.tile_pool(name="tr", bufs=4, space="PSUM"))
        ctx_p = ctx.enter_context(tc.tile_pool(name="ctx", bufs=2, space="PSUM"))

        # ---- HAM warm-up + identity, before any SWDGE prep hits Pool
        junk = sb.tile([128, 128], F32)
        nc.gpsimd.memset(junk[:], 0.0)
        warm = sc_p.tile([128, 512], F32, tag="sc", name="warm")
        for i in range(nwarm):
            nc.tensor.matmul(
                warm[:, 0:WARM_N], junk[:], junk[:, 0:WARM_N],
                start=(i == 0), stop=(i == nwarm - 1),
            )

        ident = sb.tile([128, 128], F32)
        ident16 = sb.tile([128, 128], F16)
        make_identity(nc, ident[:])
        nc.vector.tensor_copy(ident16[:], ident[:])
        zeros = sb.tile([128, KCS], F32)
        nc.gpsimd.memset(zeros[:], 0.0)

        # ---- all input DMAs up-front (SWDGE f32 -> f16 casts)
        dec_sb = {}
        enc_sb_all = {}

        def dma_dec(b):
            t = dec_p.tile([128, D], F16, tag="dec", name=f"dec{b}")
            nc.gpsimd.dma_start(t[:], dec[b])
            dec_sb[b] = t

        def dma_chunk(g, pieces=1):
            b, kc = divmod(g, KCH)
            et = enc_p.tile([128, NSUB, D], F16, tag="enc", name=f"enc{g}")
            enc_sb_all[g] = et
            # SWDGE prep cost is ~flat per instruction: full chunks keep
            # Pool light; chunk 0 is split so the head starts early
            w = D // pieces
            for i in range(pieces):
                nc.gpsimd.dma_start(
                    et[:, :, w * i : w * (i + 1)],
                    enc[b, kc * KCS : (kc + 1) * KCS, w * i : w * (i + 1)]
                    .rearrange("(n p) d -> p n d", p=128),
                )

        dma_dec(0)
        dma_chunk(0, pieces=4)
        dma_chunk(1, pieces=2)
        dma_dec(1)
        for g in range(2, NCHUNK):
            dma_chunk(g, pieces=2)

        copy_count = [0]

        def psum2sbuf(dst, src, engines="both"):
            if engines == "dve":
                nc.vector.tensor_copy(dst, src)
                return
            if copy_count[0] % 2 == 0:
                nc.vector.tensor_copy(dst, src)
            else:
                nc.scalar.copy(dst, src)
            copy_count[0] += 1

        state = {}

        def begin_batch(b):
            st = {}
            nsum = KCH
            st["negsh"] = stat_p.tile([128, 1], F32, tag="negsh", name=f"ns{b}")
            st["rm"] = stat_p.tile([128, KCH], F32, tag="rm", name=f"rm{b}")
            st["probs"] = probs_p.tile([128, Tk], F16, tag="probs", name=f"pr{b}")
            st["sums"] = stat_p.tile([128, nsum], F32, tag="sums", name=f"sm{b}")
            st["scall"] = scall_p.tile([128, Tk], F32, tag="scall", name=f"sa{b}")
            st["cps"] = [
                ctx_p.tile([128, 512], F32, tag="ctx", name=f"cps{b}_{dh}")
                for dh in range(DH)
            ]
            state[b] = st

        def emit_decT(b):
            st = state[b]
            decb = dec_sb[b]
            decT = []
            for blk in range(2):
                trt = tr_p.tile([128, 1024], F16, tag="tr", name=f"trd{b}_{blk}")[
                    :, 0:512
                ]
                for j in range(4):
                    dd = 4 * blk + j
                    nc.tensor.transpose(
                        trt[:, 128 * j : 128 * (j + 1)],
                        decb[:, 128 * dd : 128 * (dd + 1)],
                        ident16[:],
                    )
                dstT = decT_p.tile([128, 512], F16, tag="decT", name=f"dT{b}_{blk}")
                psum2sbuf(dstT[:], trt[:])
                decT.append(dstT)
            st["decT"] = decT

        def stage_a(g, with_decT=False):
            """transposes + mm1 + row-max + scores PSUM->SBUF copy."""
            b, kc = divmod(g, KCH)
            st = state[b]
            et = enc_sb_all[g]
            scores = sc_p.tile([128, KCS], F32, tag="sc", name=f"sc{g}")
            encT = {}
            pend = []

            def mm1(dd):
                nc.tensor.matmul(
                    scores[:],
                    st["decT"][dd // 4][:, 128 * (dd % 4) : 128 * (dd % 4 + 1)],
                    encT[dd][:],
                    start=(dd == 0),
                    stop=(dd == DT - 1),
                )

            for dp in range(DT // 2):
                trt = tr_p.tile([128, 1024], F16, tag="tr", name=f"tr{g}_{dp}")
                for h2 in range(2):
                    d = 2 * dp + h2
                    for n in range(NSUB):
                        nc.tensor.transpose(
                            trt[:, 512 * h2 + 128 * n : 512 * h2 + 128 * (n + 1)],
                            et[:, n, 128 * d : 128 * (d + 1)],
                            ident16[:],
                        )
                eT = encT_p.tile([128, 1024], F16, tag="encT", name=f"eT{g}_{dp}")
                psum2sbuf(eT[:], trt[:])
                encT[2 * dp] = eT[:, 0:512]
                encT[2 * dp + 1] = eT[:, 512:1024]
                pend.extend([2 * dp, 2 * dp + 1])
                if with_decT and dp == 2:
                    emit_decT(b)
                if dp >= 1 and (not with_decT or dp >= 3):
                    mm1(pend.pop(0))
                    mm1(pend.pop(0))
            while pend:
                mm1(pend.pop(0))

            nc.vector.reduce_max(st["rm"][:, kc : kc + 1], scores[:], axis=AX_X)
            nc.vector.tensor_copy(
                st["scall"][:, kc * KCS : (kc + 1) * KCS], scores[:]
            )

        def emit_negsh_exps(b, split_last=False):
            st = state[b]
            # exact shift: -(row max over all 2048)
            nc.vector.reduce_max(st["negsh"][:], st["rm"][:], axis=AX_X, negate=True)
            pieces = [(0, 512), (512, 1024), (1024, 1536)]
            if split_last:
                pieces += [(1536, 1792), (1792, 2048)]
            else:
                pieces += [(1536, 2048)]
            for i, (c0, c1) in enumerate(pieces):
                nc.scalar.activation(
                    st["probs"][:, c0:c1],
                    st["scall"][:, c0:c1],
                    EXP,
                    bias=st["negsh"][:],
                    scale=1.0,
                    accum_out=st["sums"][:, i : i + 1],
                )

        def stage_b(g, pjs=None, h=0, copy_eng="both"):
            """probsT + mm2 for k-subtiles pjs (default all 4) of chunk g."""
            b, kc = divmod(g, KCH)
            st = state[b]
            if pjs is None:
                pjs = [0, 1, 2, 3]
            trt = tr_p.tile([128, 1024], F16, tag="tr", name=f"trp{g}_{h}")[
                :, 0 : 128 * len(pjs)
            ]
            for i, j in enumerate(pjs):
                t = 4 * kc + j
                nc.tensor.transpose(
                    trt[:, 128 * i : 128 * (i + 1)],
                    st["probs"][:, 128 * t : 128 * (t + 1)],
                    ident16[:],
                )
            pT = probsT_p.tile(
                [128, 128 * len(pjs)], F16, tag="probsT", name=f"pT{g}_{h}"
            )
            psum2sbuf(pT[:], trt[:], engines=copy_eng)
            et = enc_sb_all[g]
            last = g == NCHUNK - 1
            order = (
                [(dh, i) for dh in range(DH) for i in range(len(pjs))]
                if last
                else [(dh, i) for i in range(len(pjs)) for dh in range(DH)]
            )
            for dh, i in order:
                t = 4 * kc + pjs[i]
                nc.tensor.matmul(
                    st["cps"][dh][:],
                    pT[:, 128 * i : 128 * (i + 1)],
                    et[:, pjs[i], dh * 512 : (dh + 1) * 512],
                    start=(t == 0),
                    stop=(t == 4 * KCH - 1),
                )

        def finish_batch(b):
            st = state[b]
            denom = stat_p.tile([128, 1], F32, tag="denom", name=f"dn{b}")
            nc.vector.reduce_sum(denom[:], st["sums"][:], axis=AX_X)
            rdenom = stat_p.tile([128, 1], F32, tag="rdenom", name=f"rd{b}")
            nc.vector.reciprocal(rdenom[:], denom[:])
            out_sb = outp_p.tile([128, D], F32, tag="outp", name=f"ou{b}")
            if b == BLOC - 1:
                # one full-half scale per engine (parallel), 2 DMAs on
                # separate HWDGE queues: shortest store chain
                nc.scalar.activation(
                    out_sb[:, 0:512], st["cps"][0][:], COPY, bias=0.0,
                    scale=rdenom[:],
                )
                nc.vector.tensor_scalar_mul(
                    out_sb[:, 512:1024], st["cps"][1][:], rdenom[:],
                )
                nc.sync.dma_start(out[b][:, 0:512], out_sb[:, 0:512])
                nc.scalar.dma_start(out[b][:, 512:1024], out_sb[:, 512:1024])
            else:
                nc.vector.tensor_scalar_mul(
                    out_sb[:, 0:512], st["cps"][0][:], rdenom[:],
                )
                nc.vector.tensor_scalar_mul(
                    out_sb[:, 512:1024], st["cps"][1][:], rdenom[:],
                )
                nc.sync.dma_start(out[b], out_sb[:])

        # ---- PE program
        begin_batch(0)
        emit_decT(0)
        stage_a(0)
        stage_a(1)
        stage_a(2)
        stage_a(3)
        emit_negsh_exps(0)
        begin_batch(1)
        stage_a(4, with_decT=True)
        stage_a(5)
        stage_a(6)
        stage_b(0)
        stage_a(7)
        stage_b(1)
        emit_negsh_exps(1)
        stage_b(2, copy_eng="dve")
        stage_b(3, copy_eng="dve")
        finish_batch(0)
        stage_b(4, copy_eng="dve")
        stage_b(5, copy_eng="dve")
        stage_b(6, copy_eng="dve")
        stage_b(7, copy_eng="dve")
        finish_batch(1)

    nc.compile()
    return nc


def kernel(encoder_hiddens: np.ndarray, decoder_hidden: np.ndarray) -> np.ndarray:
    enc = np.ascontiguousarray(np.asarray(encoder_hiddens, dtype=np.float32))
    dec = np.ascontiguousarray(np.asarray(decoder_hidden, dtype=np.float32))
    assert enc.shape == (B, Tk, D) and dec.shape == (B, Tq, D)

    if "nc" not in _CACHE:
        _CACHE["nc"] = _build()
    nc = _CACHE["nc"]

    from concourse.bass_utils import run_bass_kernel_spmd

    in_maps = [
        {
            "enc": enc[c * BLOC : (c + 1) * BLOC],
            "dec": dec[c * BLOC : (c + 1) * BLOC],
        }
        for c in range(CORES)
    ]
    res = None
    for attempt in range(3):
        try:
            res = run_bass_kernel_spmd(nc, in_maps, core_ids=list(range(CORES)))
            break
        except Exception:
            if attempt == 2:
                raise
            import time

            time.sleep(15)
    out = np.empty((B, Tq, D), dtype=np.float32)
    for c in range(CORES):
        out[c * BLOC : (c + 1) * BLOC] = res.results[c]["out"]
    return out


# revision 7
# speedup vs baseline: 1.0350x; 1.0030x over previous
"""Trainium2 Bass kernel for cross-attention (fp16, true-max softmax):
    scores  = dec @ enc^T            [B, Tq, Tk]
    probs   = softmax(scores, -1)
    context = probs @ enc            [B, Tq, D]

Shapes (hardcoded): enc [16, 2048, 1024] f32, dec [16, 128, 1024] f32.
Sharding: data-parallel over batch B across 8 NeuronCores (2 batches/core).

Design:
  - enc/dec stream in as fp16 via gpsimd casting DMAs (HBM still reads
    f32; SBUF holds f16) -> PE transposes run at 1.0 cyc/col instead of
    1.5, PSUM->SBUF copies halve, and mm1/mm2 run f16 x f16 -> f32 PSUM.
  - Two-phase softmax per batch: all 4 mm1 chunks first (scores copied
    PSUM -> SBUF f32), per-chunk row-max on DVE, then ONE exact shift
    bias = -rowmax(all 2048) for the exps. exp <= 1 always, so f16
    probs cannot overflow (a streamed chunk-0-max shift can be beaten
    by ~36+ in later chunks, which overflows f16's e^11 range).
  - PE order interleaves batch-1 A-phases 2:1 ahead of batch-0
    B-phases (A4 A5 A6 B0 A7 B1 | exps1 | B2 B3 finish0 B4..B7): both
    batches' max+exp chains then resolve entirely under PE fill, and
    the two-phase softmax (plain DVE copy + reduce_max; the fused
    tensor_tensor_reduce lowers to custom DVE ucode that faults the
    device) costs no PE stall.
  - All input DMAs dispatched up-front in arrival order on the SWDGE
    queue; warmup junk matmuls hold the PE HAM clock at speed until
    real work starts; identity built before the DMA preps so nothing
    queues behind them on the Pool engine.
  - Last chunk's mm2 runs fully dh-major so cps[0] closes 4 matmuls
    early: its scale+store clears the DMA engines before cps[1]'s
    store arrives. Output halves scale on ACT/DVE in parallel and
    store via sync/act HWDGE queues.
"""

import sys

sys.path.insert(0, "/opt/trn_rl_repo")

import numpy as np
from contextlib import ExitStack

import concourse.bass as bass
import concourse.tile as tile
from concourse import bacc, mybir
from concourse.masks import make_identity

F32 = mybir.dt.float32
F16 = mybir.dt.float16
EXP = mybir.ActivationFunctionType.Exp
COPY = mybir.ActivationFunctionType.Copy
AX_X = mybir.AxisListType.X

B, Tk, Tq, D = 16, 2048, 128, 1024
CORES = 8
BLOC = B // CORES          # batches per core
KCH = 4                    # k chunks per batch
KCS = Tk // KCH            # 512 k rows per chunk
NSUB = KCS // 128          # 4 k-subtiles per chunk
DT = D // 128              # 8 d-tiles
DH = D // 512              # 2 output column halves
NCHUNK = BLOC * KCH

NWARM = 29                 # junk matmuls bridging the HAM clock ramp
WARM_N = 48

_CACHE = {}


def _build(nwarm=None):
    nwarm = NWARM if nwarm is None else nwarm
    nc = bacc.Bacc("TRN2", debug=False, num_devices=CORES)
    enc = nc.dram_tensor("enc", [BLOC, Tk, D], F32, kind="ExternalInput").ap()
    dec = nc.dram_tensor("dec", [BLOC, Tq, D], F32, kind="ExternalInput").ap()
    out = nc.dram_tensor("out", [BLOC, Tq, D], F32, kind="ExternalOutput").ap()

    with tile.TileContext(nc) as tc, ExitStack() as ctx:
        sb = ctx.enter_context(tc.tile_pool(name="sb", bufs=1))
        enc_p = ctx.enter_context(tc.tile_pool(name="enc", bufs=8))
        encT_p = ctx.enter_context(tc.tile_pool(name="encT", bufs=14))
        dec_p = ctx.enter_context(tc.tile_pool(name="dec", bufs=2))
        decT_p = ctx.enter_context(tc.tile_pool(name="decT", bufs=4))
        probs_p = ctx.enter_context(tc.tile_pool(name="probs", bufs=2))
        probsT_p = ctx.enter_context(tc.tile_pool(name="probsT", bufs=8))
        scall_p = ctx.enter_context(tc.tile_pool(name="scall", bufs=2))
        outp_p = ctx.enter_context(tc.tile_pool(name="outp", bufs=2))
        stat_p = ctx.enter_context(tc.tile_pool(name="stat", bufs=4))
        sc_p = ctx.enter_context(tc.tile_pool(name="sc", bufs=2, space="PSUM"))
        tr_p = ctx.enter_context(tc.tile_pool(name="tr", bufs=4, space="PSUM"))
        ctx_p = ctx.enter_context(tc.tile_pool(name="ctx", bufs=2, space="PSUM"))

        # ---- HAM warm-up + identity, before any SWDGE prep hits Pool
        junk = sb.tile([128, 128], F32)
        nc.gpsimd.memset(junk[:], 0.0)
        warm = sc_p.tile([128, 512], F32, tag="sc", name="warm")
        for i in range(nwarm):
            nc.tensor.matmul(
                warm[:, 0:WARM_N], junk[:], junk[:, 0:WARM_N],
                start=(i == 0), stop=(i == nwarm - 1),
            )

        ident = sb.tile([128, 128], F32)
        ident16 = sb.tile([128, 128], F16)
        make_identity(nc, ident[:])
        nc.vector.tensor_copy(ident16[:], ident[:])
        zeros = sb.tile([128, KCS], F32)
        nc.gpsimd.memset(zeros[:], 0.0)

        # ---- all input DMAs up-front (SWDGE f32 -> f16 casts)
        dec_sb = {}
        enc_sb_all = {}

        def dma_dec(b):
            t = dec_p.tile([128, D], F16, tag="dec", name=f"dec{b}")
            nc.gpsimd.dma_start(t[:], dec[b])
            dec_sb[b] = t

        def dma_chunk(g, pieces=1):
            b, kc = divmod(g, KCH)
            et = enc_p.tile([128, NSUB, D], F16, tag="enc", name=f"enc{g}")
            enc_sb_all[g] = et
            # SWDGE prep cost is ~flat per instruction: full chunks keep
            # Pool light; chunk 0 is split so the head starts early
            w = D // pieces
            for i in range(pieces):
                nc.gpsimd.dma_start(
                    et[:, :, w * i : w * (i + 1)],
                    enc[b, kc * KCS : (kc + 1) * KCS, w * i : w * (i + 1)]
                    .rearrange("(n p) d -> p n d", p=128),
                )

        dma_dec(0)
        dma_chunk(0, pieces=4)
        dma_chunk(1, pieces=2)
        dma_dec(1)
        for g in range(2, NCHUNK):
            dma_chunk(g, pieces=2)

        copy_count = [0]

        def psum2sbuf(dst, src, engines="both"):
            if engines == "dve":
                nc.vector.tensor_copy(dst, src)
                return
            if copy_count[0] % 2 == 0:
                nc.vector.tensor_copy(dst, src)
            else:
                nc.scalar.copy(dst, src)
            copy_count[0] += 1

        state = {}

        def begin_batch(b):
            st = {}
            nsum = KCH
            st["negsh"] = stat_p.tile([128, 1], F32, tag="negsh", name=f"ns{b}")
            st["rm"] = stat_p.tile([128, KCH], F32, tag="rm", name=f"rm{b}")
            st["probs"] = probs_p.tile([128, Tk], F16, tag="probs", name=f"pr{b}")
            st["sums"] = stat_p.tile([128, nsum], F32, tag="sums", name=f"sm{b}")
            st["scall"] = scall_p.tile([128, Tk], F32, tag="scall", name=f"sa{b}")
            st["cps"] = [
                ctx_p.tile([128, 512], F32, tag="ctx", name=f"cps{b}_{dh}")
                for dh in range(DH)
            ]
            state[b] = st

        def emit_decT(b):
            st = state[b]
            decb = dec_sb[b]
            decT = []
            for blk in range(2):
                trt = tr_p.tile([128, 1024], F16, tag="tr", name=f"trd{b}_{blk}")[
                    :, 0:512
                ]
                for j in range(4):
                    dd = 4 * blk + j
                    nc.tensor.transpose(
                        trt[:, 128 * j : 128 * (j + 1)],
                        decb[:, 128 * dd : 128 * (dd + 1)],
                        ident16[:],
                    )
                dstT = decT_p.tile([128, 512], F16, tag="decT", name=f"dT{b}_{blk}")
                psum2sbuf(dstT[:], trt[:])
                decT.append(dstT)
            st["decT"] = decT

        def stage_a(g, with_decT=False):
            """transposes + mm1 + row-max + scores PSUM->SBUF copy."""
            b, kc = divmod(g, KCH)
            st = state[b]
            et = enc_sb_all[g]
            scores = sc_p.tile([128, KCS], F32, tag="sc", name=f"sc{g}")
            encT = {}
            pend = []

            def mm1(dd):
                nc.tensor.matmul(
                    scores[:],
                    st["decT"][dd // 4][:, 128 * (dd % 4) : 128 * (dd % 4 + 1)],
                    encT[dd][:],
                    start=(dd == 0),
                    stop=(dd == DT - 1),
                )

            for dp in range(DT // 2):
                trt = tr_p.tile([128, 1024], F16, tag="tr", name=f"tr{g}_{dp}")
                for h2 in range(2):
                    d = 2 * dp + h2
                    for n in range(NSUB):
                        nc.tensor.transpose(
                            trt[:, 512 * h2 + 128 * n : 512 * h2 + 128 * (n + 1)],
                            et[:, n, 128 * d : 128 * (d + 1)],
                            ident16[:],
                        )
                eT = encT_p.tile([128, 1024], F16, tag="encT", name=f"eT{g}_{dp}")
                psum2sbuf(eT[:], trt[:])
                encT[2 * dp] = eT[:, 0:512]
                encT[2 * dp + 1] = eT[:, 512:1024]
                pend.extend([2 * dp, 2 * dp + 1])
                if with_decT and dp == 2:
                    emit_decT(b)
                if dp >= 1 and (not with_decT or dp >= 3):
                    mm1(pend.pop(0))
                    mm1(pend.pop(0))
            while pend:
                mm1(pend.pop(0))

            nc.vector.reduce_max(st["rm"][:, kc : kc + 1], scores[:], axis=AX_X)
            nc.vector.tensor_copy(
                st["scall"][:, kc * KCS : (kc + 1) * KCS], scores[:]
            )

        def emit_negsh_exps(b, split_last=False):
            st = state[b]
            # exact shift: -(row max over all 2048)
            nc.vector.reduce_max(st["negsh"][:], st["rm"][:], axis=AX_X, negate=True)
            pieces = [(0, 512), (512, 1024), (1024, 1536)]
            if split_last:
                pieces += [(1536, 1792), (1792, 2048)]
            else:
                pieces += [(1536, 2048)]
            for i, (c0, c1) in enumerate(pieces):
                nc.scalar.activation(
                    st["probs"][:, c0:c1],
                    st["scall"][:, c0:c1],
                    EXP,
                    bias=st["negsh"][:],
                    scale=1.0,
                    accum_out=st["sums"][:, i : i + 1],
                )

        def stage_b(g, pjs=None, h=0, copy_eng="both"):
            """probsT + mm2 for k-subtiles pjs (default all 4) of chunk g."""
            b, kc = divmod(g, KCH)
            st = state[b]
            if pjs is None:
                pjs = [0, 1, 2, 3]
            trt = tr_p.tile([128, 1024], F16, tag="tr", name=f"trp{g}_{h}")[
                :, 0 : 128 * len(pjs)
            ]
            for i, j in enumerate(pjs):
                t = 4 * kc + j
                nc.tensor.transpose(
                    trt[:, 128 * i : 128 * (i + 1)],
                    st["probs"][:, 128 * t : 128 * (t + 1)],
                    ident16[:],
                )
            pT = probsT_p.tile(
                [128, 128 * len(pjs)], F16, tag="probsT", name=f"pT{g}_{h}"
            )
            psum2sbuf(pT[:], trt[:], engines=copy_eng)
            et = enc_sb_all[g]
            last = g == NCHUNK - 1
            order = (
                [(dh, i) for dh in range(DH) for i in range(len(pjs))]
                if last
                else [(dh, i) for i in range(len(pjs)) for dh in range(DH)]
            )
            for dh, i in order:
                t = 4 * kc + pjs[i]
                nc.tensor.matmul(
                    st["cps"][dh][:],
                    pT[:, 128 * i : 128 * (i + 1)],
                    et[:, pjs[i], dh * 512 : (dh + 1) * 512],
                    start=(t == 0),
                    stop=(t == 4 * KCH - 1),
                )

        def finish_batch(b):
            st = state[b]
            denom = stat_p.tile([128, 1], F32, tag="denom", name=f"dn{b}")
            nc.vector.reduce_sum(denom[:], st["sums"][:], axis=AX_X)
            rdenom = stat_p.tile([128, 1], F32, tag="rdenom", name=f"rd{b}")
            nc.vector.reciprocal(rdenom[:], denom[:])
            out_sb = outp_p.tile([128, D], F32, tag="outp", name=f"ou{b}")
            if b == BLOC - 1:
                # one full-half scale per engine (parallel), 2 DMAs on
                # separate HWDGE queues: shortest store chain
                nc.scalar.activation(
                    out_sb[:, 0:512], st["cps"][0][:], COPY, bias=0.0,
                    scale=rdenom[:],
                )
                nc.vector.tensor_scalar_mul(
                    out_sb[:, 512:1024], st["cps"][1][:], rdenom[:],
                )
                nc.sync.dma_start(out[b][:, 0:512], out_sb[:, 0:512])
                nc.scalar.dma_start(out[b][:, 512:1024], out_sb[:, 512:1024])
            else:
                nc.vector.tensor_scalar_mul(
                    out_sb[:, 0:512], st["cps"][0][:], rdenom[:],
                )
                nc.vector.tensor_scalar_mul(
                    out_sb[:, 512:1024], st["cps"][1][:], rdenom[:],
                )
                nc.sync.dma_start(out[b], out_sb[:])

        # ---- PE program
        begin_batch(0)
        emit_decT(0)
        stage_a(0)
        stage_a(1)
        stage_a(2)
        stage_a(3)
        emit_negsh_exps(0)
        begin_batch(1)
        stage_a(4, with_decT=True)
        stage_a(5)
        stage_a(6)
        stage_b(0)
        stage_a(7)
        stage_b(1)
        emit_negsh_exps(1)
        stage_b(2, copy_eng="dve")
        stage_b(3, copy_eng="dve")
        finish_batch(0)
        stage_b(4, copy_eng="dve")
        stage_b(5, copy_eng="dve")
        stage_b(6, copy_eng="dve")
        stage_b(7, copy_eng="dve")
        finish_batch(1)

    nc.compile()
    return nc


def kernel(encoder_hiddens: np.ndarray, decoder_hidden: np.ndarray) -> np.ndarray:
    enc = np.ascontiguousarray(np.asarray(encoder_hiddens, dtype=np.float32))
    dec = np.ascontiguousarray(np.asarray(decoder_hidden, dtype=np.float32))
    assert enc.shape == (B, Tk, D) and dec.shape == (B, Tq, D)

    if "nc" not in _CACHE:
        _CACHE["nc"] = _build()
    nc = _CACHE["nc"]

    from concourse.bass_utils import run_bass_kernel_spmd

    in_maps = [
        {
            "enc": enc[c * BLOC : (c + 1) * BLOC],
            "dec": dec[c * BLOC : (c + 1) * BLOC],
        }
        for c in range(CORES)
    ]
    res = None
    for attempt in range(3):
        try:
            res = run_bass_kernel_spmd(nc, in_maps, core_ids=list(range(CORES)))
            break
        except Exception:
            if attempt == 2:
                raise
            import time

            time.sleep(15)
    out = np.empty((B, Tq, D), dtype=np.float32)
    for c in range(CORES):
        out[c * BLOC : (c + 1) * BLOC] = res.results[c]["out"]
    return out
